# revision 15
# baseline (speedup 1.0000x reference)
"""Cross-attention Trainium2 kernel (Bass/Tile), data-parallel over batch on 8 cores.

Reference computation per batch element b (no 1/sqrt(d) scaling):
    Q = S2[b] @ Wq            [N2, E]
    K = S1[b] @ Wk            [N1, E]
    V = S1[b] @ Wv            [N1, E]
    A = softmax(Q @ K^T, -1)  [N2, N1]
    out[b] = (A @ V) @ Wo + bo  [N2, D]

Key algebraic reduction (inner dim E=1024 exceeds query dim D=512, so both
E-wide contractions collapse through associativity):
    scores = S2 (Wq Wk^T) S1^T          with M   = Wq @ Wk^T   [D, D] (host)
    out    = A (S1 (Wv Wo) + bo)        with WVO = Wv @ Wo     [D, D] (host)
bo folds into the value rows exactly because softmax rows sum to 1.
Per-core MACs drop from 12.9G to 5.4G; the output projection disappears.

Device layout (feature dims on SBUF partitions; host supplies transposes):
    phase A: VW[m, d] = S1^T-tiles^T @ WVO + bo  -> bf16, SBUF-resident
    phase B per 512-query chunk:
      TT[d, n] = M^T-tiles^T @ S2T chunk (f32r)
      scoresT tiles [m-part, n-free] = S1-tiles^T @ TT (f32r) -> exp to bf16
      (no max subtraction: |score| <= ~70, exp fits fp32/bf16 range)
      row sums via DVE partial-sum tree + ONE ones-matmul -> reciprocal ->
      gpsimd partition_broadcast; UT[d, n] accumulates VW^T @ E in PSUM,
      normalized by 1/sumexp during eviction -> DRAM [D, N2]; host transposes.
"""
import sys

sys.path.insert(0, "/opt/trn_rl_repo")

import numpy as np
from contextlib import ExitStack

P = 128
N_CORES = 8
B = 8          # batch (one element per core)
NQ = 2048      # queries (N2)
NK = 2048      # keys (N1)
D = 512        # query/cross dim
EI = 1024      # inner dim (eliminated on device by associativity)
CHUNK = 512    # query-chunk width (moving free dim)

_cache = {}


def _build(nq=NQ, nk=NK):
    import concourse.tile as tile
    from concourse import bacc, mybir
    from concourse.bass_isa import ReduceOp

    F32 = mybir.dt.float32
    F32R = mybir.dt.float32r
    BF16 = mybir.dt.bfloat16
    Exp = mybir.ActivationFunctionType.Exp

    n_chunks = nq // CHUNK
    m_tiles = nk // P        # key tiles of 128
    d_tiles = D // P         # 4
    m_chunks = nk // CHUNK   # phase-A key chunks

    nc = bacc.Bacc("TRN2", target_bir_lowering=False, debug=False)

    S1T = nc.dram_tensor("S1T", [D, nk], F32R, kind="ExternalInput").ap()
    S2T = nc.dram_tensor("S2T", [D, nq], F32R, kind="ExternalInput").ap()
    M = nc.dram_tensor("M", [D, D], F32R, kind="ExternalInput").ap()
    WVO = nc.dram_tensor("WVO", [D, D], F32R, kind="ExternalInput").ap()
    BO = nc.dram_tensor("BO", [1, D], F32, kind="ExternalInput").ap()
    OUT = nc.dram_tensor("OUT", [D, nq], F32, kind="ExternalOutput").ap()

    with tile.TileContext(nc) as tc, ExitStack() as ctx, \
            nc.allow_low_precision(reason="f32r/bf16 staging for matmul operands"):
        const = ctx.enter_context(tc.tile_pool(name="const", bufs=1))
        w_pool = ctx.enter_context(tc.tile_pool(name="w_pool", bufs=1))
        ps_mm = ctx.enter_context(tc.tile_pool(name="ps_mm", bufs=4, space="PSUM"))
        ps_ut = ctx.enter_context(tc.tile_pool(name="ps_ut", bufs=4, space="PSUM"))

        # constants (bo rides the gpsimd queue; emitted after M below)
        bo_row = const.tile([1, D], F32, name="bo_row")
        bo_bc = const.tile([P, D], F32, name="bo_bc")

        # persistent tensors
        m_t = w_pool.tile([P, d_tiles, D], F32R, name="m_t")       # M[d', d]
        s1_res = w_pool.tile([P, d_tiles, nk], F32R, name="s1_res")  # S1T
        vw_t = w_pool.tile([P, m_tiles, D], BF16, name="vw_t")     # S1@WVO+bo

        # ---------------- Phase B pools (declared early: TT(0) precedes
        # phase-A VW in the PE stream to cover the S1/WVO load latency) ----
        s2_pool = ctx.enter_context(tc.tile_pool(name="s2_pool", bufs=2))
        tt_pool = ctx.enter_context(tc.tile_pool(name="tt_pool", bufs=2))
        e_pool = ctx.enter_context(tc.tile_pool(name="e_pool", bufs=m_tiles + 2))
        out_pool = ctx.enter_context(tc.tile_pool(name="out_pool", bufs=4))
        misc = ctx.enter_context(tc.tile_pool(name="misc", bufs=2))

        def emit_tt(c, s2_t=None):
            """Compute TT[d, n] = M^T @ S2T chunk (f32r)."""
            if s2_t is None:
                csl = slice(c * CHUNK, (c + 1) * CHUNK)
                s2_t = s2_pool.tile(
                    [P, d_tiles, CHUNK], F32R, name="s2_t", tag="s2")
                nc.sync.dma_start(
                    s2_t[:], S2T[:, csl].rearrange("(t p) n -> p t n", p=P))
            tt_t = tt_pool.tile([P, d_tiles, CHUNK], F32R, name="tt_t", tag="tt")
            for db in range(d_tiles):
                acct = ps_mm.tile([P, CHUNK], F32, name="accT", tag="mm")
                for dt_ in range(d_tiles):
                    nc.tensor.matmul(
                        acct[:],
                        m_t[:, dt_, db * P:(db + 1) * P],
                        s2_t[:, dt_, :],
                        start=(dt_ == 0), stop=(dt_ == d_tiles - 1),
                    )
                nc.vector.tensor_copy(tt_t[:, db, :], acct[:])
            return tt_t

        # ---------------- Phase A: TT(0), then VW = S1 @ WVO + bo --------
        with tc.tile_pool(name="pa_w", bufs=1) as pa_w, \
                nc.named_scope("phaseA"):
            wvo_t = pa_w.tile([P, d_tiles, D], F32R, name="wvo_t")
            wvo_r = WVO.rearrange("(t p) d -> p t d", p=P)
            m_r = M.rearrange("(t p) d -> p t d", p=P)

            # startup loads spread over three DMA queues so TT(0)'s and
            # VW's operands land as early as possible:
            #   sync:   S2(0) per d-tile, then S1 chunks 0-1
            #   scalar: WVO per d-tile, then S1 chunks 2-3
            #   gpsimd: M per d-tile, then bo
            s2_0 = s2_pool.tile([P, d_tiles, CHUNK], F32R, name="s2_t", tag="s2")
            s2_r0 = S2T[:, 0:CHUNK].rearrange("(t p) n -> p t n", p=P)
            for dt_ in range(d_tiles):
                nc.sync.dma_start(s2_0[:, dt_, :], s2_r0[:, dt_, :])
                nc.gpsimd.dma_start(m_t[:, dt_, :], m_r[:, dt_, :])
                nc.scalar.dma_start(wvo_t[:, dt_, :], wvo_r[:, dt_, :])
            nc.gpsimd.dma_start(bo_row[:], BO[:, :])
            nc.gpsimd.partition_broadcast(bo_bc[:], bo_row[:])

            s1_r = [
                S1T[:, mc * CHUNK:(mc + 1) * CHUNK].rearrange(
                    "(t p) m -> p t m", p=P)
                for mc in range(m_chunks)
            ]
            for dt_ in range(d_tiles):
                nc.sync.dma_start(
                    s1_res[:, dt_, 0:CHUNK], s1_r[0][:, dt_, :])
            nc.sync.dma_start(
                s1_res[:, :, 1 * CHUNK:2 * CHUNK], s1_r[1])
            for mc in range(2, m_chunks):
                nc.scalar.dma_start(
                    s1_res[:, :, mc * CHUNK:(mc + 1) * CHUNK], s1_r[mc])

            tt_cur = emit_tt(0, s2_t=s2_0)

            for mc in range(m_chunks):
                for ml in range(CHUNK // P):
                    mt = mc * (CHUNK // P) + ml
                    accv = ps_mm.tile([P, D], F32, name="accV", tag="mm")
                    for dt_ in range(d_tiles):
                        nc.tensor.matmul(
                            accv[:],
                            s1_res[:, dt_, mt * P:(mt + 1) * P],
                            wvo_t[:, dt_, :],
                            start=(dt_ == 0), stop=(dt_ == d_tiles - 1),
                        )
                    nc.vector.tensor_add(vw_t[:, mt, :], accv[:], bo_bc[:])

        for c in range(n_chunks):
          with nc.named_scope(f"chunk{c}"):
            csl = slice(c * CHUNK, (c + 1) * CHUNK)
            tt_t = tt_cur

            # scoresT tiles + exp + DVE partial-sum tree over m-tiles
            sum_acc = misc.tile([P, CHUNK], BF16, name="sum_acc", tag="sacc")
            e_list = []
            for mt in range(m_tiles):
                acc_s = ps_mm.tile([P, CHUNK], F32, name="acc_s", tag="mm")
                for dt_ in range(d_tiles):
                    nc.tensor.matmul(
                        acc_s[:],
                        s1_res[:, dt_, mt * P:(mt + 1) * P],
                        tt_t[:, dt_, :],
                        start=(dt_ == 0), stop=(dt_ == d_tiles - 1),
                    )
                e_t = e_pool.tile([P, CHUNK], BF16, name="e_t", tag="e")
                nc.scalar.activation(e_t[:], acc_s[:], Exp)
                e_list.append(e_t)
                if mt == 0:
                    nc.vector.tensor_copy(sum_acc[:], e_t[:])
                else:
                    nc.vector.tensor_add(sum_acc[:], sum_acc[:], e_t[:])

            # prefetch next chunk's TT while the softmax chain runs on DVE
            if c + 1 < n_chunks:
                tt_cur = emit_tt(c + 1)

            # gpsimd all-reduce contracts the 128 partitions of sum_acc and
            # broadcasts the result; reciprocal gives the softmax scale
            sums_bc = misc.tile([P, CHUNK], F32, name="sums_bc", tag="sbc")
            nc.gpsimd.partition_all_reduce(
                sums_bc[:], sum_acc[:], P, ReduceOp.add)
            bc = misc.tile([P, CHUNK], F32, name="bc", tag="bc")
            nc.vector.reciprocal(bc[:], sums_bc[:])

            # UT[d, n] = sum_mt VW^T @ E per d-block, normalized + stored
            # as soon as each block's accumulation completes
            for db in range(d_tiles):
                ut = ps_ut.tile([P, CHUNK], F32, name="ut", tag="ut")
                for mt in range(m_tiles):
                    nc.tensor.matmul(
                        ut[:],
                        vw_t[:, mt, db * P:(db + 1) * P],
                        e_list[mt][:],
                        start=(mt == 0), stop=(mt == m_tiles - 1),
                    )
                o_sb = out_pool.tile([P, CHUNK], F32, name="o_sb", tag="osb")
                nc.vector.tensor_mul(o_sb[:], ut[:], bc[:])
                nc.sync.dma_start(OUT[db * P:(db + 1) * P, csl], o_sb[:])

    nc.compile()
    return nc


def _get_nc(nq=NQ, nk=NK):
    key = (nq, nk)
    if key not in _cache:
        _cache[key] = _build(nq, nk)
    return _cache[key]


def kernel(S1, S2, Wq, Wk, Wv, Wo, bo, _trace=False):
    from concourse.bass_utils import run_bass_kernel_spmd

    S1 = np.asarray(S1, np.float32)
    S2 = np.asarray(S2, np.float32)
    b, nk, _ = S1.shape
    _, nq, _ = S2.shape
    nc = _get_nc(nq, nk)

    # host-side weight collapse (exact up to fp64 rounding)
    Wq = np.asarray(Wq, np.float64)
    Wk = np.asarray(Wk, np.float64)
    Wv = np.asarray(Wv, np.float64)
    Wo = np.asarray(Wo, np.float64)
    m = np.ascontiguousarray((Wq @ Wk.T).astype(np.float32))      # [D, D]
    wvo = np.ascontiguousarray((Wv @ Wo).astype(np.float32))      # [D, D]
    bo_r = np.ascontiguousarray(
        np.asarray(bo, np.float32).reshape(1, D))

    in_maps = []
    for i in range(b):
        in_maps.append({
            "S1T": np.ascontiguousarray(S1[i].T),
            "S2T": np.ascontiguousarray(S2[i].T),
            "M": m, "WVO": wvo, "BO": bo_r,
        })

    res = run_bass_kernel_spmd(nc, in_maps, list(range(b)), trace=_trace)
    out = np.stack([np.asarray(res.results[i]["OUT"]).T for i in range(b)])
    if _trace:
        kernel.last_result = res
    return np.ascontiguousarray(out.astype(np.float32))
